# revision 14
# baseline (speedup 1.0000x reference)
"""Trainium2 Bass kernel for the HOI relation model.

8 cores data-parallel over batch (2 images/core). The DMA wire (one
sync-engine HWDGE queue at ~414 GB/s aggregate) is the fixed cost;
everything else is scheduled to hide under it:

  wire order: mask | imgA features (2 groups) | w1 | w2 | imgB
  features (groups shrinking to 1 chunk at the end) | w3+bias.

Image A's post-pooling chain (transpose epilogue, layer 1 with layer 2
pipelined per 128-col block, layer 3 chunk 0 + its store) runs on the
tensor engine between image B's pooling groups, i.e. under B's DMA
stream.  Only B's chain, layer-3 chunks 1-2 and their stores remain in
the tail, and the trailing B feature groups are small ([.., 4, 2, 1]
chunks) so pooling tracks arrival instead of waiting on coarse group
semaphores.  The dense PE run from layer-1(A) onward also un-throttles
the HAM clock gate so the tail executes at 2.4 GHz.

Per image:
  1. ROI mean pooling over a host-packed stream of UNION pixels:
     per-chunk matmuls with the mask stationary [128,32] (1/area
     folded in) and features moving in three N=256 channel thirds that
     run CONCURRENTLY in PE column strips 0/1/2 into one [96,256] PSUM.
  2. PE-transpose pooled -> pooledT [768, 32], pipelined per strip.
  3. Layer 1 factorized: relu(pair(h,o) @ w1 + b1) = relu(A(h)+B(o)+b1);
     the 8x24 pair expansion happens AFTER the matmul, per 128-row h1
     block: psum copy (vector) -> broadcast add (vector/gpsimd) ->
     fused relu+bias (scalar).  The layer-2 matmuls for block mc are
     emitted right after block mc's activation (kc-pipelined).
  4. Layer 3 + per-chunk bias adds; chunk 0 (image A pairs) is stored
     mid-stream, chunks 1-2 at the end.

All DRAM arrays are pre-packed host-side partition-major so every
dma_start is 128 descriptors of multi-KB contiguous runs.  The program
is specialized on (kch0, kch1) and cached; images are assigned to
cores sorted by union size so slot KCHs pad minimally.

Host does only O(B*D + union-gather) prep: rasterization, argsort
order, 1/area, dtype casts, layout packing, shard/gather.
"""

import numpy as np
import ml_dtypes

import concourse.bass as bass
import concourse.mybir as mybir
import concourse.tile as tile
from concourse import bacc
from concourse.bass_utils import run_bass_kernel_spmd

N_CORES = 8
B, D, C = 16, 32, 768
NH, NO = 8, 24
NPAIR = NH * NO              # 192 pairs per image
GRID = 64                    # feature grid (896 / 14)
BL = 2                       # images per core
CGA = 7                      # chunks per DMA group, image A stream
CPX = 128                    # pixels per chunk
H1, H2, H3 = 512, 256, 117
M = BL * NPAIR               # 384 pair rows per core

F32 = mybir.dt.float32
BF16 = mybir.dt.bfloat16
BF = ml_dtypes.bfloat16
RELU = mybir.ActivationFunctionType.Relu
COPY = mybir.ActivationFunctionType.Copy

_PROGRAMS = {}               # (kch0, kch1) -> compiled Bacc


def _groups_a(kch):
    """Plain groups of CGA chunks for the image-A stream."""
    out = []
    o = 0
    while o < kch:
        g = min(CGA, kch - o)
        out.append((o, g))
        o += g
    return out


def _groups_b(kch):
    """Groups of <=6 with a shrinking tail (.., 4, 2, 1) so the last
    pooling work is gated by small DMA completions."""
    sizes = []
    rem = kch
    while rem > 7:
        sizes.append(6)
        rem -= 6
    if rem >= 4:
        sizes += [rem - 3, 2, 1]
    elif rem == 3:
        sizes += [2, 1]
    elif rem == 2:
        sizes += [1, 1]
    else:
        sizes += [1]
    out = []
    o = 0
    for g in sizes:
        out.append((o, g))
        o += g
    return out


PIPE = True


def _build_program(kchs):
    kch0, kch1 = kchs
    ktot = kch0 + kch1
    add = mybir.AluOpType.add

    nc = bacc.Bacc("TRN2", target_bir_lowering=False, debug=False,
                   num_devices=N_CORES)
    # flat partition-major pixel stream: img0 chunks then img1 chunks
    feat = nc.declare_dram_parameter("feat", [CPX, ktot * C], BF16,
                                     isOutput=False)
    # mask blocks [128, ktot*D] (1/area folded in) + identity [96,96]
    maskR = nc.declare_dram_parameter("maskR", [128, ktot * D + 96], BF16,
                                      isOutput=False)
    w1 = nc.declare_dram_parameter("w1", [128, 12 * H1], BF16, isOutput=False)
    w2 = nc.declare_dram_parameter("w2", [128, 4 * H2], BF16, isOutput=False)
    w3 = nc.declare_dram_parameter("w3", [128, 2 * H3], BF16, isOutput=False)
    bias = nc.declare_dram_parameter("bias", [128, 128], F32, isOutput=False)
    out = nc.declare_dram_parameter("out", [128, 3 * H3], F32, isOutput=True)

    gsA = _groups_a(kch0)
    gsB = _groups_b(kch1)
    maxg = max(g for _, g in gsA + gsB)

    with tile.TileContext(nc) as tc:
        with (
            tc.tile_pool(name="singles", bufs=1) as singles,
            tc.tile_pool(name="featp", bufs=len(gsA) + len(gsB)) as featp,
            tc.tile_pool(name="work", bufs=1) as work,
            tc.tile_pool(name="tmp", bufs=3) as tmpp,
            tc.tile_pool(name="pps", bufs=1, space="PSUM") as pps,
            tc.tile_pool(name="l2ps", bufs=1, space="PSUM") as l2ps,
            tc.tile_pool(name="mps", bufs=2, space="PSUM") as mps,
            tc.tile_pool(name="lastp", bufs=1, space="PSUM") as lastp,
        ):
            m_sb = singles.tile([128, ktot * D + 96], BF16, tag="mask")
            w1_sb = singles.tile([128, 12 * H1], BF16, tag="w1")
            w2_sb = singles.tile([128, 4 * H2], BF16, tag="w2")
            w3_sb = singles.tile([128, 2 * H3], BF16, tag="w3")
            bias_sb = singles.tile([128, 128], F32, tag="bias")
            ident96 = m_sb[0:96, ktot * D:ktot * D + 96]
            b1_sb = bias_sb[:, 0:4]
            b2_sb = bias_sb[:, 4:6]
            b3_sb = bias_sb[:, 6:6 + H3]

            pooledT = work.tile([128, 6, BL, D], BF16, tag="pooledT")
            x1T = work.tile([128, 4, M], BF16, tag="x1T")
            x2T = work.tile([128, 2, M], BF16, tag="x2T")
            o_sb = work.tile([128, 3, H3], F32, tag="osb")
            ps3 = pps.tile([128, 3, H3], F32, tag="ps3")

            # ---- wire: mask | A feats | w1 | w2,w3,bias | B feats ----
            nc.sync.dma_start(out=m_sb, in_=maskR[:, :])
            fA = []
            for o, gl in gsA:
                f_sb = featp.tile([CPX, maxg * C], BF16, tag="f")
                nc.sync.dma_start(out=f_sb[:, 0:gl * C],
                                  in_=feat[:, o * C:(o + gl) * C])
                fA.append(f_sb)
            nc.sync.dma_start(out=w1_sb, in_=w1[:, :])
            nc.sync.dma_start(out=w2_sb, in_=w2[:, :])
            nc.sync.dma_start(out=w3_sb, in_=w3[:, :])
            nc.sync.dma_start(out=bias_sb, in_=bias[:, :])
            fB = []
            for o, gl in gsB:
                f_sb = featp.tile([CPX, maxg * C], BF16, tag="f")
                nc.sync.dma_start(
                    out=f_sb[:, 0:gl * C],
                    in_=feat[:, (kch0 + o) * C:(kch0 + o + gl) * C])
                fB.append(f_sb)

            def pool_group(ps_j, koff, klast, o, gl, f_sb):
                # accumulation group covers chunks 0..klast (inclusive)
                for gc in range(gl):
                    kk = o + gc
                    mk = m_sb[0:CPX, (koff + kk) * D:(koff + kk + 1) * D]
                    # channel thirds run concurrently in PE column
                    # strips 0/1/2 (column tiling, M=32)
                    for s in range(3):
                        nc.tensor.matmul(
                            ps_j[32 * s:32 * s + 32, :], mk,
                            f_sb[:, gc * C + 256 * s:gc * C + 256 * (s + 1)],
                            start=(kk == 0), stop=(kk == klast),
                            tile_position=(0, 32 * s))

            def pool_last_T(j, koff, chunks):
                # trailing chunks folded directly in transposed form:
                # T[c,d] += f_chunk[:,c].T @ mask[:,d]  (6 128-col
                # stationaries, FWL) then one strided add into pooledT
                ps_l = lastp.tile([128, 6, D], F32, tag="lastT")
                n = len(chunks)
                # c6 outer: accumulation groups in one bank must be
                # sequential, not interleaved
                for c6 in range(6):
                    for ci, (kk, f_sb, gc) in enumerate(chunks):
                        mk = m_sb[0:CPX, (koff + kk) * D:(koff + kk + 1) * D]
                        nc.tensor.matmul(
                            ps_l[:, c6, :],
                            f_sb[:, gc * C + c6 * 128:gc * C + (c6 + 1) * 128],
                            mk, start=(ci == 0), stop=(ci == n - 1))
                nc.vector.tensor_tensor(pooledT[:, :, j, :],
                                        pooledT[:, :, j, :], ps_l,
                                        op=add)

            def epilogue(j, ps_j):
                # band s (partitions 32s:32s+32) holds channels 256s:256s+256
                # copies and transposes pipelined per strip
                pooled = tmpp.tile([96, 256], BF16, tag=f"pool{j}")
                for s in range(3):
                    eng = nc.vector if s != 1 else nc.scalar
                    if s == 1:
                        nc.scalar.activation(pooled[32:64, :],
                                             ps_j[32:64, :], COPY)
                    else:
                        nc.vector.tensor_copy(pooled[32 * s:32 * s + 32, :],
                                              ps_j[32 * s:32 * s + 32, :])
                    for h in range(2):
                        cc = 2 * s + h
                        ps_t = mps.tile([128, D], BF16, tag="mm")
                        nc.tensor.transpose(
                            ps_t,
                            pooled[32 * s:32 * s + 32,
                                   h * 128:(h + 1) * 128],
                            ident96[32 * s:32 * s + 32, 32 * s:32 * s + 32])
                        if cc == 1 or cc == 4:
                            nc.scalar.activation(pooledT[:, cc, j, :],
                                                 ps_t, COPY)
                        else:
                            nc.vector.tensor_copy(pooledT[:, cc, j, :], ps_t)

            def l1_mc(j, mc, ps2, pipelined=True):
                # ps2: pair of [128, NPAIR] psum tiles (separate banks --
                # interleaved accumulation groups may not share a bank)
                """Layer-1 block mc for image j, with the layer-2
                matmuls for contraction chunk kc==mc pipelined in."""
                ps_ab = mps.tile([128, D], F32, tag="mm")
                for kc in range(6):
                    nc.tensor.matmul(
                        ps_ab[:, 0:NH],
                        w1_sb[:, kc * H1 + mc * 128:kc * H1 + (mc + 1) * 128],
                        pooledT[:, kc, j, 0:NH],
                        start=(kc == 0), stop=(kc == 5))
                for kc in range(6):
                    nc.tensor.matmul(
                        ps_ab[:, NH:D],
                        w1_sb[:, (6 + kc) * H1 + mc * 128:(6 + kc) * H1 + (mc + 1) * 128],
                        pooledT[:, kc, j, NH:D],
                        start=(kc == 0), stop=(kc == 5))
                ab_sb = tmpp.tile([128, D], BF16, tag=f"ab{j}")
                nc.vector.tensor_copy(ab_sb, ps_ab)
                pre = tmpp.tile([128, NH, NO], BF16, tag=f"pre{j}")
                h = NH // 2
                a0 = ab_sb[:, 0:h][:, :, None].broadcast_to([128, h, NO])
                a1 = ab_sb[:, h:NH][:, :, None].broadcast_to([128, h, NO])
                b_bc = ab_sb[:, NH:D][:, None, :].broadcast_to([128, h, NO])
                nc.vector.tensor_tensor(pre[:, 0:h], a0, b_bc, op=add)
                nc.gpsimd.tensor_tensor(pre[:, h:NH], a1, b_bc, op=add)
                dst = x1T[:, mc, j * NPAIR:(j + 1) * NPAIR] \
                    .rearrange("p (i k) -> p i k", i=NH)
                nc.scalar.activation(dst, pre, RELU, bias=b1_sb[:, mc:mc + 1])
                if not pipelined:
                    if mc == 3:
                        l2_all(j, ps2)
                    return
                # layer 2, contraction chunk kc = mc, both 128-col halves
                for m2 in range(2):
                    nc.tensor.matmul(
                        ps2[m2],
                        w2_sb[:, mc * H2 + m2 * 128:mc * H2 + (m2 + 1) * 128],
                        x1T[:, mc, j * NPAIR:(j + 1) * NPAIR],
                        start=(mc == 0), stop=(mc == 3))
                if mc == 3:
                    for m2 in range(2):
                        nc.scalar.activation(
                            x2T[:, m2, j * NPAIR:(j + 1) * NPAIR],
                            ps2[m2], RELU, bias=b2_sb[:, m2:m2 + 1])

            def l2_all(j, ps2):
                for m2 in range(2):
                    for kc in range(4):
                        nc.tensor.matmul(
                            ps2[m2],
                            w2_sb[:, kc * H2 + m2 * 128:kc * H2 + (m2 + 1) * 128],
                            x1T[:, kc, j * NPAIR:(j + 1) * NPAIR],
                            start=(kc == 0), stop=(kc == 3))
                    nc.scalar.activation(
                        x2T[:, m2, j * NPAIR:(j + 1) * NPAIR],
                        ps2[m2], RELU, bias=b2_sb[:, m2:m2 + 1])

            def l3_store(m3, store=True):
                for kc in range(2):
                    nc.tensor.matmul(ps3[:, m3, :],
                                     x2T[:, kc, m3 * 128:(m3 + 1) * 128],
                                     w3_sb[:, kc * H3:(kc + 1) * H3],
                                     start=(kc == 0), stop=(kc == 1))
                nc.vector.tensor_tensor(o_sb[:, m3, :], ps3[:, m3, :],
                                        b3_sb, op=add)
                if store:
                    nc.sync.dma_start(out=out[:, m3 * H3:(m3 + 1) * H3],
                                      in_=o_sb[:, m3, :])

            # ---- image A pooling ----
            ps_0 = pps.tile([96, 256], F32, tag="ps0")
            for gi, (o, gl) in enumerate(gsA):
                pool_group(ps_0, 0, kch0 - 1, o, gl, fA[gi])
            epilogue(0, ps_0)
            # ---- image A MLP interleaved with image B pooling ----
            ps2_0a = l2ps.tile([128, NPAIR], F32, tag="l2m0")
            ps2_0b = l2ps.tile([128, NPAIR], F32, tag="l2m1")
            ps2_0 = (ps2_0a, ps2_0b)
            ps_1 = pps.tile([96, 256], F32, tag="ps1")
            pieces = [lambda: l1_mc(0, 0, ps2_0, PIPE), lambda: l1_mc(0, 1, ps2_0, PIPE),
                      lambda: l1_mc(0, 2, ps2_0, PIPE), lambda: l1_mc(0, 3, ps2_0, PIPE),
                      lambda: l3_store(0)]
            # emission order: 2 pieces, pool g, 2 pieces, pool g, ...
            nflip = sum(gl for _, gl in gsB[-2:])
            klast_bulk = kch1 - 1 - nflip
            pi = 0
            for gi, (o, gl) in enumerate(gsB[:-2]):
                while pi < len(pieces) and pi < 2 * (gi + 1):
                    pieces[pi]()
                    pi += 1
                pool_group(ps_1, kch0, klast_bulk, o, gl, fB[gi])
            while pi < len(pieces):
                pieces[pi]()
                pi += 1
            # ---- image B chain (the tail) ----
            epilogue(1, ps_1)
            flip_chunks = []
            for gi in (len(gsB) - 2, len(gsB) - 1):
                o, gl = gsB[gi]
                for gc in range(gl):
                    flip_chunks.append((o + gc, fB[gi], gc))
            pool_last_T(1, kch0, flip_chunks)
            ps2_1a = l2ps.tile([128, NPAIR], F32, tag="l2m0")
            ps2_1b = l2ps.tile([128, NPAIR], F32, tag="l2m1")
            ps2_1 = (ps2_1a, ps2_1b)
            for mc in range(4):
                l1_mc(1, mc, ps2_1, PIPE)
            l3_store(1, store=False)
            l3_store(2, store=False)
            nc.sync.dma_start(out=out[:, H3:3 * H3], in_=o_sb[:, 1:3, :])
    nc.compile()
    return nc


def _get_program(kchs):
    if kchs not in _PROGRAMS:
        _PROGRAMS[kchs] = _build_program(kchs)
    return _PROGRAMS[kchs]


def _preprocess(boxes, scores):
    """Box corners (reference's floor math), sorted det order, 1/area,
    and per-image union pixel coverage."""
    cx, cy, bw, bh = boxes[..., 0], boxes[..., 1], boxes[..., 2], boxes[..., 3]
    x1 = np.floor((cx - bw / 2) * GRID).astype(np.int64)
    y1 = np.floor((cy - bh / 2) * GRID).astype(np.int64)
    x2 = np.floor((cx + bw / 2) * GRID).astype(np.int64)
    y2 = np.floor((cy + bh / 2) * GRID).astype(np.int64)
    hidx = np.argsort(-scores[:, :NH], axis=1, kind="stable")
    oidx = np.argsort(-scores[:, NH:], axis=1, kind="stable") + NH
    perm = np.concatenate([hidx, oidx], axis=1)                     # [B, D]
    g = np.arange(GRID)
    rows = (g[None, None, :] >= y1[..., None]) & (g[None, None, :] < y2[..., None])
    cols = (g[None, None, :] >= x1[..., None]) & (g[None, None, :] < x2[..., None])
    rows = np.take_along_axis(rows, perm[..., None], axis=1)        # [B, D, 64]
    cols = np.take_along_axis(cols, perm[..., None], axis=1)
    area = rows.sum(-1) * cols.sum(-1)                              # [B, D]
    cover = np.einsum('bdy,bdx->byx', rows, cols) > 0               # [B, 64, 64]
    return rows, cols, cover, (1.0 / area).astype(np.float32)


_LAST_META = {}




def _make_in_maps(features, boxes, scores, w1, b1, w2, b2, w3, b3):
    features = np.asarray(features, np.float32).reshape(B, GRID, GRID, C)
    rows, cols, cover, inva = _preprocess(np.asarray(boxes, np.float32),
                                          np.asarray(scores, np.float32))
    pys = [np.nonzero(cover[b]) for b in range(B)]
    pcount = np.array([len(p[0]) for p in pys])
    kch_img = (-(-pcount // CPX)).astype(int)
    order = np.argsort(-kch_img, kind="stable")
    # slot 0 = the 8 largest images, slot 1 = the rest (minimal padding)
    pairs = [(order[c], order[B - 1 - c]) for c in range(N_CORES)]
    kchs = (int(kch_img[order[0]]), int(kch_img[order[N_CORES]]))
    _LAST_META["kchs"] = kchs
    _LAST_META["pairs"] = pairs
    kch0, kch1 = kchs
    ktot = kch0 + kch1

    w1R = np.ascontiguousarray(
        np.asarray(w1, np.float32).reshape(12, 128, H1)
        .transpose(1, 0, 2).reshape(128, 12 * H1)).astype(BF)
    w2R = np.ascontiguousarray(
        np.asarray(w2, np.float32).reshape(4, 128, H2)
        .transpose(1, 0, 2).reshape(128, 4 * H2)).astype(BF)
    w3R = np.ascontiguousarray(
        np.asarray(w3, np.float32).reshape(2, 128, H3)
        .transpose(1, 0, 2).reshape(128, 2 * H3)).astype(BF)
    biasR = np.zeros((128, 128), np.float32)
    biasR[:, 0:4] = np.asarray(b1, np.float32).reshape(4, 128).T
    biasR[:, 4:6] = np.asarray(b2, np.float32).reshape(2, 128).T
    biasR[:, 6:6 + H3] = np.asarray(b3, np.float32)[None, :]

    in_maps = []
    for a, bidx in pairs:
        fpad = np.zeros((ktot * CPX, C), np.float32)
        mpad = np.zeros((ktot * 128, D), np.float32)
        for j, (bi, koff, kc) in enumerate(((a, 0, kch0), (bidx, kch0, kch1))):
            yy, xx = pys[bi]
            p = len(yy)
            pi = np.arange(p)
            fpad[koff * CPX + pi] = features[bi][yy, xx]
            # mask[pix, d] = (rows & cols) / area_d  (mean via matmul)
            mpad[(koff + pi // CPX) * 128 + pi % CPX] = \
                (rows[bi][:, yy] & cols[bi][:, xx]).T * inva[bi][None, :]
        featR = np.ascontiguousarray(
            fpad.reshape(ktot, CPX, C).transpose(1, 0, 2)
            .reshape(CPX, ktot * C)).astype(BF)
        maskR = np.zeros((128, ktot * D + 96), np.float32)
        maskR[:, :ktot * D] = \
            mpad.reshape(ktot, 128, D).transpose(1, 0, 2).reshape(128, -1)
        maskR[0:96, ktot * D:] = np.eye(96, dtype=np.float32)
        in_maps.append({
            "feat": featR,
            "maskR": np.ascontiguousarray(maskR).astype(BF),
            "w1": w1R, "w2": w2R, "w3": w3R, "bias": biasR,
        })
    return in_maps


def _run(in_maps, trace=False, **kw):
    nc = _get_program(_LAST_META["kchs"])
    return run_bass_kernel_spmd(nc, in_maps, core_ids=list(range(N_CORES)),
                                trace=trace, **kw)


def kernel(features, boxes, scores, w1, b1, w2, b2, w3, b3, labels):
    in_maps = _make_in_maps(features, boxes, scores, w1, b1, w2, b2, w3, b3)
    res = _run(in_maps, trace=False)
    out = np.empty((B, NPAIR, H3), np.float32)
    for c, (a, bidx) in enumerate(_LAST_META["pairs"]):
        r = res.results[c]["out"].reshape(128, 3, H3) \
            .transpose(1, 0, 2).reshape(M, H3)
        out[a] = r[0:NPAIR]
        out[bidx] = r[NPAIR:M]
    return np.ascontiguousarray(out)


# revision 15
# speedup vs baseline: 1.1164x; 1.1164x over previous
"""Trainium2 Bass kernel for the HOI relation model.

8 cores data-parallel over batch (2 images/core). The DMA wire (one
sync-engine HWDGE queue at ~414 GB/s aggregate) is the fixed cost;
everything else is scheduled to hide under it:

  wire order: mask | imgA features (2 groups) | w1 | w2 | imgB
  features (groups shrinking to 1 chunk at the end) | w3+bias.

Image A's post-pooling chain (transpose epilogue, layer 1 with layer 2
pipelined per 128-col block, layer 3 chunk 0 + its store) runs on the
tensor engine between image B's pooling groups, i.e. under B's DMA
stream.  Only B's chain, layer-3 chunks 1-2 and their stores remain in
the tail, and the trailing B feature groups are small ([.., 4, 2, 1]
chunks) so pooling tracks arrival instead of waiting on coarse group
semaphores.  The dense PE run from layer-1(A) onward also un-throttles
the HAM clock gate so the tail executes at 2.4 GHz.

Per image:
  1. ROI mean pooling over a host-packed stream of UNION pixels:
     per-chunk matmuls with the mask stationary [128,32] (1/area
     folded in) and features moving in three N=256 channel thirds that
     run CONCURRENTLY in PE column strips 0/1/2 into one [96,256] PSUM.
  2. PE-transpose pooled -> pooledT [768, 32], pipelined per strip.
  3. Layer 1 factorized: relu(pair(h,o) @ w1 + b1) = relu(A(h)+B(o)+b1);
     the 8x24 pair expansion happens AFTER the matmul, per 128-row h1
     block: psum copy (vector) -> broadcast add (vector/gpsimd) ->
     fused relu+bias (scalar).  The layer-2 matmuls for block mc are
     emitted right after block mc's activation (kc-pipelined).
  4. Layer 3 + per-chunk bias adds; chunk 0 (image A pairs) is stored
     mid-stream, chunks 1-2 at the end.

All DRAM arrays are pre-packed host-side partition-major so every
dma_start is 128 descriptors of multi-KB contiguous runs.  The program
is specialized on (kch0, kch1) and cached; images are assigned to
cores sorted by union size so slot KCHs pad minimally.

Host does only O(B*D + union-gather) prep: rasterization, argsort
order, 1/area, dtype casts, layout packing, shard/gather.
"""

import numpy as np
import ml_dtypes

import concourse.bass as bass
import concourse.mybir as mybir
import concourse.tile as tile
from concourse import bacc
from concourse.bass_utils import run_bass_kernel_spmd

N_CORES = 8
B, D, C = 16, 32, 768
NH, NO = 8, 24
NPAIR = NH * NO              # 192 pairs per image
GRID = 64                    # feature grid (896 / 14)
BL = 2                       # images per core
CGA = 7                      # chunks per DMA group, image A stream
CPX = 128                    # pixels per chunk
H1, H2, H3 = 512, 256, 117
M = BL * NPAIR               # 384 pair rows per core

F32 = mybir.dt.float32
BF16 = mybir.dt.bfloat16
BF = ml_dtypes.bfloat16
RELU = mybir.ActivationFunctionType.Relu
COPY = mybir.ActivationFunctionType.Copy

_PROGRAMS = {}               # (kch0, kch1) -> compiled Bacc


def _groups_a(kch):
    """Plain groups of CGA chunks for the image-A stream."""
    out = []
    o = 0
    while o < kch:
        g = min(CGA, kch - o)
        out.append((o, g))
        o += g
    return out


def _groups_b(kch):
    """Groups of <=6 with a shrinking tail (.., 4, 2, 1) so the last
    pooling work is gated by small DMA completions."""
    sizes = []
    rem = kch
    while rem > 7:
        sizes.append(6)
        rem -= 6
    if rem >= 4:
        sizes += [rem - 3, 2, 1]
    elif rem == 3:
        sizes += [2, 1]
    elif rem == 2:
        sizes += [1, 1]
    else:
        sizes += [1]
    out = []
    o = 0
    for g in sizes:
        out.append((o, g))
        o += g
    return out


PIPE = True


def _build_program(kchs):
    kch0, kch1 = kchs
    ktot = kch0 + kch1
    add = mybir.AluOpType.add

    nc = bacc.Bacc("TRN2", target_bir_lowering=False, debug=False,
                   num_devices=N_CORES)
    # flat partition-major pixel stream: img0 chunks then img1 chunks
    feat = nc.declare_dram_parameter("feat", [CPX, ktot * C], BF16,
                                     isOutput=False)
    # mask blocks [128, ktot*D] (1/area folded in) + identity [96,96]
    maskR = nc.declare_dram_parameter("maskR", [128, ktot * D + 96], BF16,
                                      isOutput=False)
    w1 = nc.declare_dram_parameter("w1", [128, 12 * H1], BF16, isOutput=False)
    w2 = nc.declare_dram_parameter("w2", [128, 4 * H2], BF16, isOutput=False)
    w3 = nc.declare_dram_parameter("w3", [128, 2 * H3], BF16, isOutput=False)
    bias = nc.declare_dram_parameter("bias", [128, 128], F32, isOutput=False)
    out = nc.declare_dram_parameter("out", [128, 3 * H3], F32, isOutput=True)

    gsA = _groups_a(kch0)
    gsB = _groups_b(kch1)
    maxg = max(g for _, g in gsA + gsB)

    with tile.TileContext(nc) as tc:
        with (
            tc.tile_pool(name="singles", bufs=1) as singles,
            tc.tile_pool(name="featp", bufs=len(gsA) + len(gsB)) as featp,
            tc.tile_pool(name="work", bufs=1) as work,
            tc.tile_pool(name="tmp", bufs=3) as tmpp,
            tc.tile_pool(name="pps", bufs=1, space="PSUM") as pps,
            tc.tile_pool(name="l2ps", bufs=1, space="PSUM") as l2ps,
            tc.tile_pool(name="mps", bufs=2, space="PSUM") as mps,
            tc.tile_pool(name="lastp", bufs=1, space="PSUM") as lastp,
        ):
            m_sb = singles.tile([128, ktot * D + 96], BF16, tag="mask")
            w1_sb = singles.tile([128, 12 * H1], BF16, tag="w1")
            w2_sb = singles.tile([128, 4 * H2], BF16, tag="w2")
            w3_sb = singles.tile([128, 2 * H3], BF16, tag="w3")
            bias_sb = singles.tile([128, 128], F32, tag="bias")
            ident96 = m_sb[0:96, ktot * D:ktot * D + 96]
            b1_sb = bias_sb[:, 0:4]
            b2_sb = bias_sb[:, 4:6]
            b3_sb = bias_sb[:, 6:6 + H3]

            pooledT = work.tile([128, 6, BL, D], BF16, tag="pooledT")
            x1T = work.tile([128, 4, M], BF16, tag="x1T")
            x2T = work.tile([128, 2, M], BF16, tag="x2T")
            o_sb = work.tile([128, 3, H3], F32, tag="osb")
            ps3 = pps.tile([128, 3, H3], F32, tag="ps3")

            # ---- wire: mask | A feats | w1 | w2,w3,bias | B feats ----
            nc.sync.dma_start(out=m_sb, in_=maskR[:, :])
            fA = []
            for o, gl in gsA:
                f_sb = featp.tile([CPX, maxg * C], BF16, tag="f")
                nc.sync.dma_start(out=f_sb[:, 0:gl * C],
                                  in_=feat[:, o * C:(o + gl) * C])
                fA.append(f_sb)
            nc.sync.dma_start(out=w1_sb, in_=w1[:, :])
            nc.sync.dma_start(out=w2_sb, in_=w2[:, :])
            nc.sync.dma_start(out=w3_sb, in_=w3[:, :])
            nc.sync.dma_start(out=bias_sb, in_=bias[:, :])
            fB = []
            for o, gl in gsB:
                f_sb = featp.tile([CPX, maxg * C], BF16, tag="f")
                nc.sync.dma_start(
                    out=f_sb[:, 0:gl * C],
                    in_=feat[:, (kch0 + o) * C:(kch0 + o + gl) * C])
                fB.append(f_sb)

            def pool_group(ps_j, koff, klast, o, gl, f_sb):
                # accumulation group covers chunks 0..klast (inclusive)
                for gc in range(gl):
                    kk = o + gc
                    mk = m_sb[0:CPX, (koff + kk) * D:(koff + kk + 1) * D]
                    # channel thirds run concurrently in PE column
                    # strips 0/1/2 (column tiling, M=32)
                    for s in range(3):
                        nc.tensor.matmul(
                            ps_j[32 * s:32 * s + 32, :], mk,
                            f_sb[:, gc * C + 256 * s:gc * C + 256 * (s + 1)],
                            start=(kk == 0), stop=(kk == klast),
                            tile_position=(0, 32 * s))

            def pool_last_T(j, koff, chunks):
                # trailing chunks folded directly in transposed form:
                # T[c,d] += f_chunk[:,c].T @ mask[:,d]  (6 128-col
                # stationaries, FWL) then one strided add into pooledT
                ps_l = lastp.tile([128, 6, D], F32, tag="lastT")
                n = len(chunks)
                # c6 outer: accumulation groups in one bank must be
                # sequential, not interleaved
                for c6 in range(6):
                    for ci, (kk, f_sb, gc) in enumerate(chunks):
                        mk = m_sb[0:CPX, (koff + kk) * D:(koff + kk + 1) * D]
                        nc.tensor.matmul(
                            ps_l[:, c6, :],
                            f_sb[:, gc * C + c6 * 128:gc * C + (c6 + 1) * 128],
                            mk, start=(ci == 0), stop=(ci == n - 1))
                nc.vector.tensor_tensor(pooledT[:, :, j, :],
                                        pooledT[:, :, j, :], ps_l,
                                        op=add)

            def epilogue(j, ps_j):
                # band s (partitions 32s:32s+32) holds channels 256s:256s+256
                # copies and transposes pipelined per strip
                pooled = tmpp.tile([96, 256], BF16, tag=f"pool{j}")
                for s in range(3):
                    eng = nc.vector if s != 1 else nc.scalar
                    if s == 1:
                        nc.scalar.activation(pooled[32:64, :],
                                             ps_j[32:64, :], COPY)
                    else:
                        nc.vector.tensor_copy(pooled[32 * s:32 * s + 32, :],
                                              ps_j[32 * s:32 * s + 32, :])
                    for h in range(2):
                        cc = 2 * s + h
                        ps_t = mps.tile([128, D], BF16, tag="mm")
                        nc.tensor.transpose(
                            ps_t,
                            pooled[32 * s:32 * s + 32,
                                   h * 128:(h + 1) * 128],
                            ident96[32 * s:32 * s + 32, 32 * s:32 * s + 32])
                        if cc == 1 or cc == 4:
                            nc.scalar.activation(pooledT[:, cc, j, :],
                                                 ps_t, COPY)
                        else:
                            nc.vector.tensor_copy(pooledT[:, cc, j, :], ps_t)

            def l1_mc(j, mc, ps2, pipelined=True):
                # ps2: pair of [128, NPAIR] psum tiles (separate banks --
                # interleaved accumulation groups may not share a bank)
                """Layer-1 block mc for image j, with the layer-2
                matmuls for contraction chunk kc==mc pipelined in."""
                ps_ab = mps.tile([128, D], F32, tag="mm")
                for kc in range(6):
                    nc.tensor.matmul(
                        ps_ab[:, 0:NH],
                        w1_sb[:, kc * H1 + mc * 128:kc * H1 + (mc + 1) * 128],
                        pooledT[:, kc, j, 0:NH],
                        start=(kc == 0), stop=(kc == 5))
                for kc in range(6):
                    nc.tensor.matmul(
                        ps_ab[:, NH:D],
                        w1_sb[:, (6 + kc) * H1 + mc * 128:(6 + kc) * H1 + (mc + 1) * 128],
                        pooledT[:, kc, j, NH:D],
                        start=(kc == 0), stop=(kc == 5))
                ab_sb = tmpp.tile([128, D], BF16, tag=f"ab{j}")
                nc.vector.tensor_copy(ab_sb, ps_ab)
                pre = tmpp.tile([128, NH, NO], BF16, tag=f"pre{j}")
                h = NH // 2
                a0 = ab_sb[:, 0:h][:, :, None].broadcast_to([128, h, NO])
                a1 = ab_sb[:, h:NH][:, :, None].broadcast_to([128, h, NO])
                b_bc = ab_sb[:, NH:D][:, None, :].broadcast_to([128, h, NO])
                nc.vector.tensor_tensor(pre[:, 0:h], a0, b_bc, op=add)
                nc.gpsimd.tensor_tensor(pre[:, h:NH], a1, b_bc, op=add)
                dst = x1T[:, mc, j * NPAIR:(j + 1) * NPAIR] \
                    .rearrange("p (i k) -> p i k", i=NH)
                nc.scalar.activation(dst, pre, RELU, bias=b1_sb[:, mc:mc + 1])
                if not pipelined:
                    if mc == 3:
                        l2_all(j, ps2)
                    return
                # layer 2, contraction chunk kc = mc, both 128-col halves
                for m2 in range(2):
                    nc.tensor.matmul(
                        ps2[m2],
                        w2_sb[:, mc * H2 + m2 * 128:mc * H2 + (m2 + 1) * 128],
                        x1T[:, mc, j * NPAIR:(j + 1) * NPAIR],
                        start=(mc == 0), stop=(mc == 3))
                if mc == 3:
                    for m2 in range(2):
                        nc.scalar.activation(
                            x2T[:, m2, j * NPAIR:(j + 1) * NPAIR],
                            ps2[m2], RELU, bias=b2_sb[:, m2:m2 + 1])

            def l2_all(j, ps2):
                for m2 in range(2):
                    for kc in range(4):
                        nc.tensor.matmul(
                            ps2[m2],
                            w2_sb[:, kc * H2 + m2 * 128:kc * H2 + (m2 + 1) * 128],
                            x1T[:, kc, j * NPAIR:(j + 1) * NPAIR],
                            start=(kc == 0), stop=(kc == 3))
                    nc.scalar.activation(
                        x2T[:, m2, j * NPAIR:(j + 1) * NPAIR],
                        ps2[m2], RELU, bias=b2_sb[:, m2:m2 + 1])

            def l3_store(m3, store=True):
                for kc in range(2):
                    nc.tensor.matmul(ps3[:, m3, :],
                                     x2T[:, kc, m3 * 128:(m3 + 1) * 128],
                                     w3_sb[:, kc * H3:(kc + 1) * H3],
                                     start=(kc == 0), stop=(kc == 1))
                nc.vector.tensor_tensor(o_sb[:, m3, :], ps3[:, m3, :],
                                        b3_sb, op=add)
                if store:
                    nc.sync.dma_start(out=out[:, m3 * H3:(m3 + 1) * H3],
                                      in_=o_sb[:, m3, :])

            # ---- image A pooling ----
            ps_0 = pps.tile([96, 256], F32, tag="ps0")
            for gi, (o, gl) in enumerate(gsA):
                pool_group(ps_0, 0, kch0 - 1, o, gl, fA[gi])
            epilogue(0, ps_0)
            # ---- image A MLP interleaved with image B pooling ----
            ps2_0a = l2ps.tile([128, NPAIR], F32, tag="l2m0")
            ps2_0b = l2ps.tile([128, NPAIR], F32, tag="l2m1")
            ps2_0 = (ps2_0a, ps2_0b)
            ps_1 = pps.tile([96, 256], F32, tag="ps1")
            pieces = [lambda: l1_mc(0, 0, ps2_0, PIPE), lambda: l1_mc(0, 1, ps2_0, PIPE),
                      lambda: l1_mc(0, 2, ps2_0, PIPE), lambda: l1_mc(0, 3, ps2_0, PIPE),
                      lambda: l3_store(0)]
            # emission order: 2 pieces, pool g, 2 pieces, pool g, ...
            nflip = sum(gl for _, gl in gsB[-1:])
            klast_bulk = kch1 - 1 - nflip
            pi = 0
            for gi, (o, gl) in enumerate(gsB[:-1]):
                while pi < len(pieces) and pi < 2 * (gi + 1):
                    pieces[pi]()
                    pi += 1
                pool_group(ps_1, kch0, klast_bulk, o, gl, fB[gi])
            while pi < len(pieces):
                pieces[pi]()
                pi += 1
            # ---- image B chain (the tail) ----
            epilogue(1, ps_1)
            flip_chunks = []
            for gi in (len(gsB) - 1,):
                o, gl = gsB[gi]
                for gc in range(gl):
                    flip_chunks.append((o + gc, fB[gi], gc))
            pool_last_T(1, kch0, flip_chunks)
            ps2_1a = l2ps.tile([128, NPAIR], F32, tag="l2m0")
            ps2_1b = l2ps.tile([128, NPAIR], F32, tag="l2m1")
            ps2_1 = (ps2_1a, ps2_1b)
            for mc in range(4):
                l1_mc(1, mc, ps2_1, PIPE)
            l3_store(1, store=False)
            l3_store(2, store=False)
            nc.sync.dma_start(out=out[:, H3:3 * H3], in_=o_sb[:, 1:3, :])
    nc.compile()
    return nc


def _get_program(kchs):
    if kchs not in _PROGRAMS:
        _PROGRAMS[kchs] = _build_program(kchs)
    return _PROGRAMS[kchs]


def _preprocess(boxes, scores):
    """Box corners (reference's floor math), sorted det order, 1/area,
    and per-image union pixel coverage."""
    cx, cy, bw, bh = boxes[..., 0], boxes[..., 1], boxes[..., 2], boxes[..., 3]
    x1 = np.floor((cx - bw / 2) * GRID).astype(np.int64)
    y1 = np.floor((cy - bh / 2) * GRID).astype(np.int64)
    x2 = np.floor((cx + bw / 2) * GRID).astype(np.int64)
    y2 = np.floor((cy + bh / 2) * GRID).astype(np.int64)
    hidx = np.argsort(-scores[:, :NH], axis=1, kind="stable")
    oidx = np.argsort(-scores[:, NH:], axis=1, kind="stable") + NH
    perm = np.concatenate([hidx, oidx], axis=1)                     # [B, D]
    g = np.arange(GRID)
    rows = (g[None, None, :] >= y1[..., None]) & (g[None, None, :] < y2[..., None])
    cols = (g[None, None, :] >= x1[..., None]) & (g[None, None, :] < x2[..., None])
    rows = np.take_along_axis(rows, perm[..., None], axis=1)        # [B, D, 64]
    cols = np.take_along_axis(cols, perm[..., None], axis=1)
    area = rows.sum(-1) * cols.sum(-1)                              # [B, D]
    cover = np.einsum('bdy,bdx->byx', rows, cols) > 0               # [B, 64, 64]
    return rows, cols, cover, (1.0 / area).astype(np.float32)


_LAST_META = {}




def _make_in_maps(features, boxes, scores, w1, b1, w2, b2, w3, b3):
    features = np.asarray(features, np.float32).reshape(B, GRID, GRID, C)
    rows, cols, cover, inva = _preprocess(np.asarray(boxes, np.float32),
                                          np.asarray(scores, np.float32))
    pys = [np.nonzero(cover[b]) for b in range(B)]
    pcount = np.array([len(p[0]) for p in pys])
    kch_img = (-(-pcount // CPX)).astype(int)
    order = np.argsort(-kch_img, kind="stable")
    # slot 0 = the 8 largest images, slot 1 = the rest (minimal padding)
    pairs = [(order[c], order[B - 1 - c]) for c in range(N_CORES)]
    kchs = (int(kch_img[order[0]]), int(kch_img[order[N_CORES]]))
    _LAST_META["kchs"] = kchs
    _LAST_META["pairs"] = pairs
    kch0, kch1 = kchs
    ktot = kch0 + kch1

    w1R = np.ascontiguousarray(
        np.asarray(w1, np.float32).reshape(12, 128, H1)
        .transpose(1, 0, 2).reshape(128, 12 * H1)).astype(BF)
    w2R = np.ascontiguousarray(
        np.asarray(w2, np.float32).reshape(4, 128, H2)
        .transpose(1, 0, 2).reshape(128, 4 * H2)).astype(BF)
    w3R = np.ascontiguousarray(
        np.asarray(w3, np.float32).reshape(2, 128, H3)
        .transpose(1, 0, 2).reshape(128, 2 * H3)).astype(BF)
    biasR = np.zeros((128, 128), np.float32)
    biasR[:, 0:4] = np.asarray(b1, np.float32).reshape(4, 128).T
    biasR[:, 4:6] = np.asarray(b2, np.float32).reshape(2, 128).T
    biasR[:, 6:6 + H3] = np.asarray(b3, np.float32)[None, :]

    in_maps = []
    for a, bidx in pairs:
        fpad = np.zeros((ktot * CPX, C), np.float32)
        mpad = np.zeros((ktot * 128, D), np.float32)
        for j, (bi, koff, kc) in enumerate(((a, 0, kch0), (bidx, kch0, kch1))):
            yy, xx = pys[bi]
            p = len(yy)
            pi = np.arange(p)
            fpad[koff * CPX + pi] = features[bi][yy, xx]
            # mask[pix, d] = (rows & cols) / area_d  (mean via matmul)
            mpad[(koff + pi // CPX) * 128 + pi % CPX] = \
                (rows[bi][:, yy] & cols[bi][:, xx]).T * inva[bi][None, :]
        featR = np.ascontiguousarray(
            fpad.reshape(ktot, CPX, C).transpose(1, 0, 2)
            .reshape(CPX, ktot * C)).astype(BF)
        maskR = np.zeros((128, ktot * D + 96), np.float32)
        maskR[:, :ktot * D] = \
            mpad.reshape(ktot, 128, D).transpose(1, 0, 2).reshape(128, -1)
        maskR[0:96, ktot * D:] = np.eye(96, dtype=np.float32)
        in_maps.append({
            "feat": featR,
            "maskR": np.ascontiguousarray(maskR).astype(BF),
            "w1": w1R, "w2": w2R, "w3": w3R, "bias": biasR,
        })
    return in_maps


def _run(in_maps, trace=False, **kw):
    nc = _get_program(_LAST_META["kchs"])
    return run_bass_kernel_spmd(nc, in_maps, core_ids=list(range(N_CORES)),
                                trace=trace, **kw)


def kernel(features, boxes, scores, w1, b1, w2, b2, w3, b3, labels):
    in_maps = _make_in_maps(features, boxes, scores, w1, b1, w2, b2, w3, b3)
    res = _run(in_maps, trace=False)
    out = np.empty((B, NPAIR, H3), np.float32)
    for c, (a, bidx) in enumerate(_LAST_META["pairs"]):
        r = res.results[c]["out"].reshape(128, 3, H3) \
            .transpose(1, 0, 2).reshape(M, H3)
        out[a] = r[0:NPAIR]
        out[bidx] = r[NPAIR:M]
    return np.ascontiguousarray(out)
